# revision 13
# baseline (speedup 1.0000x reference)
"""Multi-head attention (strictly-upper-triangular mask variant) on 8 TRN2 cores.

Reference math (B=4, S=2048, D=512, H=8, A=64):
    q/k/v = per-head projections of query/key/value           [B,H,S,A]
    scores = q @ k^T / sqrt(A), lower triangle (incl diag) masked to -1e9
    out = concat_heads(softmax(scores) @ v) @ Wo + bo         [B,S,D]

Sharding: 8 cores = 4 batches x 2 interleaved q-tile sets.  Core c handles
batch b=c//2, q-tiles g = 2*i + (c%2) for i in 0..7 (128 rows each).  Every
core computes all 8 heads for its 1024 query rows; no collectives — the host
gather is a row-interleave concat.

Device-side design:
  * Q/K projections run in fp8e4 with DoubleRow perf mode (operands
    [K, 2, M]/[K, 2, N] contract two K=128 subtiles per pass — 2x the bf16
    column rate).  Score errors only perturb softmax weights (ratio-
    protected), so fp8 is safe here; weights are host-scaled x32 into
    e4m3's normal range and the combined 1/(32*32*sqrt(A)) is folded into
    the EXP activation's scale.  The value path (V projection, AV, output
    projection) stays bf16: rows attending to 1-2 keys emit essentially a
    raw v row, so e4m3's ~3% quantum there would exceed the error budget.
  * Scores are computed transposed (S^T[k,q]) in strips whose width tracks
    the causal boundary; strips pack into 6 exact 1536-wide PSUM bins so
    EXP runs as 6 big activations per head.  exp needs no max-subtraction
    (scores are O(1) bounded).
  * V is produced directly in natural [k,a] layout by swapping matmul
    roles (stationary = v^T data chunk, moving = Wv), eliminating PE
    transposes; a 64-wide ones block between the two heads' V blocks makes
    the AV matmul emit 64 replicated softmax denominators per head.
  * k-bias dropped (softmax shift invariance), v-bias folded into the
    output bias on host (softmax rows sum to 1), so K/V evictions are
    plain copies.
  * Triangular masks are 0/1 bf16 multiplies on the otherwise-idle GpSimd
    engine (last 128 columns of each key-chunk strip).
  * P^T strip storage ping-pongs between two SBUF tiles by head parity so
    head h+1's exp can overwrite while head h's AV still reads.

The single fully-masked query row (q = S-1, uniform attention in the
reference) comes back wrong from the device and is recomputed exactly on
the host during the gather.
"""

import numpy as np
import ml_dtypes

B, S, D, H, A = 4, 2048, 512, 8, 64
P = 128
NQ = 1024          # q rows per core
NQT = 8            # q tiles per core
NKC = 16           # k chunks
NPAIR = 4          # head pairs
BF = ml_dtypes.bfloat16
E4 = ml_dtypes.float8_e4m3

WSC = 32.0         # host scale on Wq/Wk (into e4m3 normal range)
EXP_SCALE = 1.0 / (WSC * WSC * 8.0)   # 2^-13: undo q,k weight scales + 1/sqrt(A)

# strip widths / offsets for the transposed-score layout (64-row q tiles
# interleaved between the 2 cores of a batch: core takes tiles t = 2i+pair)
WKC = [64 * (kc + 1) for kc in range(NKC)]
SOFF = np.concatenate([[0], np.cumsum(WKC)]).tolist()
PT_TOTAL = SOFF[-1]  # 8704
BINW = 1536
BIN_EDGE = list(range(0, PT_TOTAL, BINW)) + [PT_TOTAL]
NBINS = len(BIN_EDGE) - 1  # 6 (last bin 1024)

_cache = {}


def _split512(a, b):
    """Split [a,b) at multiples of 512 (PSUM bank boundaries)."""
    out = []
    while a < b:
        nxt = min(b, (a // 512 + 1) * 512)
        out.append((a, nxt))
        a = nxt
    return out


def _build():
    if "nc" in _cache:
        return _cache["nc"]

    import concourse.bacc as bacc
    import concourse.mybir as mybir
    import concourse.tile as tile

    F32 = mybir.dt.float32
    BF16 = mybir.dt.bfloat16
    FP8 = mybir.dt.float8e4
    MULT = mybir.AluOpType.mult
    EXP = mybir.ActivationFunctionType.Exp
    DR = mybir.MatmulPerfMode.DoubleRow

    nc = bacc.Bacc("TRN2", target_bir_lowering=False, debug=False, num_devices=8)

    # batched inputs: few large DMAs, Q-projection-critical first
    inA_d = nc.dram_tensor("inA", [P, 2048 + 4 * NQ], FP8, kind="ExternalInput")
    inB_d = nc.dram_tensor("inB", [P, 2048 + 4 * S], FP8, kind="ExternalInput")
    inC_d = nc.dram_tensor("inC", [P, 2048 + 4 * S], BF16, kind="ExternalInput")
    inD_d = nc.dram_tensor("inD", [P, 2048 + 64], BF16, kind="ExternalInput")
    bq_d = nc.dram_tensor("bq8", [P, 4], F32, kind="ExternalInput")
    bo_d = nc.dram_tensor("bo", [1, D], BF16, kind="ExternalInput")
    ones_d = nc.dram_tensor("ones1", [1, P], BF16, kind="ExternalInput")
    out_d = nc.dram_tensor("out", [NQ, D], F32, kind="ExternalOutput")

    # score-strip segments per exp bin: bin g -> [(kc, a0, a1), ...] global offsets
    bins = [[] for _ in range(NBINS)]
    for kc in range(NKC):
        for (a0, a1) in _split512(SOFF[kc], SOFF[kc] + WKC[kc]):
            g = a0 // BINW
            assert a1 <= BIN_EDGE[g + 1], (kc, a0, a1)
            bins[g].append((kc, a0, a1))

    with tile.TileContext(nc) as tc:
        with (
            tc.tile_pool(name="cst", bufs=1) as cst,
            tc.tile_pool(name="act", bufs=1) as act,
            tc.tile_pool(name="rcp", bufs=3) as rcp,
            tc.tile_pool(name="ost", bufs=3) as ost,
            tc.tile_pool(name="stg", bufs=2, space="PSUM") as stg,
            tc.tile_pool(name="avp", bufs=2, space="PSUM") as avp,
        ):
            # ---- constant loads: batched, in first-use order ----
            inA = cst.tile([P, 2048 + 4 * NQ], FP8, tag="inA")
            inB = cst.tile([P, 2048 + 4 * S], FP8, tag="inB")
            inC = cst.tile([P, 2048 + 4 * S], BF16, tag="inC")
            inD = cst.tile([P, 2048 + 64], BF16, tag="inD")
            bq = cst.tile([P, 4], F32, tag="bq")
            bo = cst.tile([1, D], BF16, tag="bo")
            on1 = cst.tile([1, P], BF16, tag="on1")
            for t, d in [(inA, inA_d), (bq, bq_d), (inB, inB_d), (inC, inC_d),
                         (inD, inD_d), (bo, bo_d), (on1, ones_d)]:
                nc.sync.dma_start(t[:], d[:])
            wq, qT8 = inA[:, 0:2048], inA[:, 2048:]
            wk, kT = inB[:, 0:2048], inB[:, 2048:]
            wv, vT = inC[:, 0:2048], inC[:, 2048:]
            wo, mD = inD[:, 0:2048], inD[:, 2048:]

            QT = [act.tile([P, NQ], BF16, tag=f"QT{p}", name=f"QT{p}") for p in range(NPAIR)]
            KT = [act.tile([P, S], BF16, tag=f"KT{p}", name=f"KT{p}") for p in range(NPAIR)]
            Vn = act.tile([P, NKC * 768], BF16, tag="Vn", name="Vn")
            # ping-pong by head parity: scores(h+1) writes its exp output
            # while attn_av(h) still reads head h's
            ptall = [act.tile([P, PT_TOTAL], BF16, tag=f"pt{i}", name=f"pt{i}")
                     for i in range(2)]
            XT = act.tile([P, 4 * NQ], BF16, tag="XT", name="XT")

            wq3 = wq.rearrange("k (b m) -> k b m", b=16)
            wk3 = wk.rearrange("k (b m) -> k b m", b=16)
            wv3 = wv.rearrange("k (c n) -> k c n", c=4)
            wo3 = wo.rearrange("k (c n) -> k c n", c=4)
            qT83 = qT8.rearrange("k (c n) -> k c n", c=4)
            kT3 = kT.rearrange("k (c n) -> k c n", c=4)
            vT3 = vT.rearrange("k (c n) -> k c n", c=4)
            Vn3 = Vn[:].rearrange("p (k c) -> p k c", c=768)
            # Vn block per k-chunk: 4x [V_h0(64) | ones(64) | V_h1(64)]
            Vn5 = Vn[:].rearrange("p (k q t f) -> p k q t f", q=4, t=3, f=64)
            XT3 = XT[:].rearrange("p (c n) -> p c n", c=4)

            # denominator ones blocks (written once)
            nc.gpsimd.memset(Vn5[:, :, :, 1, :], 1.0)

            # ---- projection pieces (fp8 DoubleRow, K=256; PSUM via avp pool) ----
            def proj_q(p, qh):
                ps = avp.tile([P, 512], F32, tag="av")
                for j in range(2):
                    nc.tensor.matmul(
                        ps[:], wq3[:, 4 * p + 2 * j:4 * p + 2 * j + 2, :],
                        qT83[:, 2 * j:2 * j + 2, 512 * qh:512 * (qh + 1)],
                        start=(j == 0), stop=(j == 1), perf_mode=DR)
                nc.vector.tensor_scalar_add(
                    QT[p][:, 512 * qh:512 * (qh + 1)], ps[:], bq[:, p:p + 1])

            def proj_k(p, sh):
                ps = avp.tile([P, 512], F32, tag="av")
                for j in range(2):
                    nc.tensor.matmul(
                        ps[:], wk3[:, 4 * p + 2 * j:4 * p + 2 * j + 2, :],
                        kT3[:, 2 * j:2 * j + 2, 512 * sh:512 * (sh + 1)],
                        start=(j == 0), stop=(j == 1), perf_mode=DR)
                nc.vector.tensor_copy(KT[p][:, 512 * sh:512 * (sh + 1)], ps[:])

            # V projection, natural [k,a] layout (stationary = data)
            def proj_v(kc):
                ps = avp.tile([P, 512], F32, tag="av")
                for ch in range(4):
                    nc.tensor.matmul(
                        ps[:], vT3[:, ch, P * kc:P * (kc + 1)],
                        wv3[:, ch, :],
                        start=(ch == 0), stop=(ch == 3))
                # psum cols = (pair, head-of-pair, a64) -> Vn V-blocks
                pv = ps[:].rearrange("p (q t f) -> p q t f", q=4, t=2, f=64)
                nc.vector.tensor_copy(Vn5[:, kc, :, 0, :], pv[:, :, 0, :])
                nc.vector.tensor_copy(Vn5[:, kc, :, 2, :], pv[:, :, 1, :])

            # ---- one score bin: matmuls + exp + masks-in-bin ----
            def scores_bin(h, g):
                p, hh = h // 2, h % 2
                pt = ptall[h % 2]
                hr = slice(64 * hh, 64 * hh + 64)
                e0, e1 = BIN_EDGE[g], BIN_EDGE[g + 1]
                st = stg.tile([P, BINW], F32, tag="big")
                for (kc, a0, a1) in bins[g]:
                    nc.tensor.matmul(
                        st[:, a0 - e0:a1 - e0],
                        KT[p][hr, P * kc:P * (kc + 1)],
                        QT[p][hr, a0 - SOFF[kc]:a1 - SOFF[kc]],
                        start=True, stop=True)
                nc.scalar.activation(
                    pt[:, e0:e1], st[:, 0:e1 - e0], EXP, scale=EXP_SCALE)
                # diagonal half-tile mask: last 64 columns of each strip
                for kc in range(NKC):
                    dend = SOFF[kc] + WKC[kc]
                    if e0 < dend <= e1:
                        nc.gpsimd.tensor_tensor(
                            pt[:, dend - 64:dend], pt[:, dend - 64:dend],
                            mD, MULT)

            # ---- AV half (bf16) + normalize ----
            def av_half(h, b):
                p, hh = h // 2, h % 2
                pt = ptall[h % 2]
                hr = slice(64 * hh, 64 * hh + 64)
                po = 192 * p + 64 * hh
                orow = 0 if hh == 0 else 64
                drow = 64 - orow
                avb = avp.tile([P, 512], F32, tag="av")
                kc0 = 8 * b
                for kc in range(kc0, NKC):
                    w = min(WKC[kc], 512 * (b + 1)) - 512 * b
                    nc.tensor.matmul(
                        avb[:, 0:w],
                        Vn3[:, kc, po:po + 128],
                        pt[:, SOFF[kc] + 512 * b:SOFF[kc] + 512 * b + w],
                        start=(kc == kc0), stop=(kc == NKC - 1),
                        skip_group_check=True)
                rec = rcp.tile([64, 1024], F32, tag="rec")
                nc.vector.tensor_copy(rec[:, 0:512], avb[drow:drow + 64, :])
                nc.vector.reciprocal_approx_fast(rec[:, 512:1024], rec[:, 0:512])
                nc.vector.tensor_tensor(
                    XT3[hr, p, 512 * b:512 * (b + 1)],
                    avb[orow:orow + 64, :], rec[:, 512:1024], MULT)

            # ---- software-pipelined schedule ----
            # PE issue order interleaves each head's score bins with the
            # previous head's AV halves and the next pair's projections, so
            # the PE never drains while EXP catches up (keeps the p-state
            # clock ramped at 2.4GHz).
            for qh in range(2):
                proj_q(0, qh)
            for sh in range(4):
                proj_k(0, sh)

            # head 0: interleave V projection chunks between score bins
            scores_bin(0, 0)
            scores_bin(0, 1)
            vq = list(range(NKC))
            for g in range(2, NBINS):
                proj_v(vq.pop(0))
                proj_v(vq.pop(0))
                proj_v(vq.pop(0))
                scores_bin(0, g)
            while vq:
                proj_v(vq.pop(0))

            for h in range(1, H):
                p = h // 2
                xq = [lambda b=b: av_half(h - 1, b) for b in range(2)]
                if h % 2 == 1 and p + 1 < NPAIR:
                    xq += [lambda qh=qh: proj_q(p + 1, qh) for qh in range(2)]
                    xq += [lambda sh=sh: proj_k(p + 1, sh) for sh in range(4)]
                scores_bin(h, 0)
                scores_bin(h, 1)
                for g in range(2, NBINS):
                    if xq:
                        xq.pop(0)()
                    scores_bin(h, g)
                while xq:
                    xq.pop(0)()
            av_half(H - 1, 0)
            av_half(H - 1, 1)

            # ---- output projection (bf16) ----
            for i in range(NQT):
                po = avp.tile([P, D], F32, tag="av")
                for ch in range(4):
                    nc.tensor.matmul(
                        po[:], XT3[:, ch, P * i:P * (i + 1)],
                        wo3[:, ch, :],
                        start=(ch == 0), stop=False)
                nc.tensor.matmul(po[:], on1[0:1, :], bo[0:1, :],
                                 start=False, stop=True)
                ob = ost.tile([P, D], F32, tag="ob")
                nc.vector.tensor_copy(ob[:], po[:])
                nc.sync.dma_start(out_d[P * i:P * (i + 1), :], ob[:])

    nc.compile()
    _cache["nc"] = nc
    return nc


def _host_prep(query, key, value, Wq, bq, Wk, bk, Wv, bv, Wo, bo):
    """Build the 8 per-core input maps (all device-side layouts)."""
    def stack_chmin(W, scale, dt):
        # [H,D,A] -> [128, 16*128], block b=4p+ch: rows 128ch of [W_2p|W_2p+1]
        blocks = []
        for p in range(NPAIR):
            Wp = np.concatenate([W[2 * p], W[2 * p + 1]], axis=1) * scale
            for ch in range(4):
                blocks.append(Wp[P * ch:P * (ch + 1), :])
        return np.stack(blocks, 1).reshape(P, -1).astype(dt)

    def stack_pmin(W, dt):
        # [H,D,A] -> [128, 16*128], block b=4ch+p (moving operand layout)
        blocks = []
        for ch in range(4):
            for p in range(NPAIR):
                Wp = np.concatenate([W[2 * p], W[2 * p + 1]], axis=1)
                blocks.append(Wp[P * ch:P * (ch + 1), :])
        return np.stack(blocks, 1).reshape(P, -1).astype(dt)

    wq_h = stack_chmin(Wq, WSC, E4)
    wk_h = stack_chmin(Wk, WSC, E4)
    wv_h = stack_pmin(Wv, BF)
    wo_h = np.stack([Wo[P * ch:P * (ch + 1), :] for ch in range(4)], 1)
    wo_h = wo_h.reshape(P, -1).astype(BF)

    bq_h = np.stack(
        [np.concatenate([bq[2 * p], bq[2 * p + 1]]) * WSC for p in range(NPAIR)],
        1).astype(np.float32)
    # v-bias adds bv to each head's attention output (softmax rows sum to 1):
    # fold it, with bo, into the single output bias row
    bo_eff = bo + np.concatenate(list(bv)) @ Wo
    bo_h = bo_eff[None, :].astype(BF)
    ones_h = np.ones((1, P), BF)
    kl = np.arange(P)[:, None]
    ql = np.arange(64)[None, :]

    def chunked_T(x, dt):
        # [S', D] -> [128, 4*S'] with col block ch = rows 128ch of x.T
        xT = np.ascontiguousarray(x.T)  # [512, S']
        return xT.reshape(4, P, -1).transpose(1, 0, 2).reshape(P, -1).astype(dt)

    in_maps = []
    for c in range(8):
        b, pair = c // 2, c % 2
        sel = np.concatenate(
            [np.arange(64 * (2 * i + pair), 64 * (2 * i + pair) + 64)
             for i in range(16)])
        mD_h = (kl > 64 * pair + ql).astype(BF)
        m = {
            "inA": np.concatenate([wq_h, chunked_T(query[b][sel], E4)], 1),
            "inB": np.concatenate([wk_h, chunked_T(key[b], E4)], 1),
            "inC": np.concatenate([wv_h, chunked_T(value[b], BF)], 1),
            "inD": np.concatenate([wo_h, mD_h], 1),
            "bq8": bq_h, "bo": bo_h, "ones1": ones_h,
        }
        in_maps.append(m)
    return in_maps


def kernel(query, key, value, Wq, bq, Wk, bk, Wv, bv, Wo, bo):
    from concourse.bass_utils import run_bass_kernel_spmd

    args = [np.asarray(a, dtype=np.float32) for a in
            (query, key, value, Wq, bq, Wk, bk, Wv, bv, Wo, bo)]
    query, key, value, Wq, bq, Wk, bk, Wv, bv, Wo, bo = args

    nc = _build()
    in_maps = _host_prep(*args)
    res = run_bass_kernel_spmd(nc, in_maps, list(range(8)))

    out = np.empty((B, S, D), np.float32)
    for c in range(8):
        b, pair = c // 2, c % 2
        o = res.results[c]["out"]
        for i in range(16):
            g = 2 * i + pair
            out[b, 64 * g:64 * g + 64, :] = o[64 * i:64 * i + 64, :]

    # q = S-1 attends to nothing -> reference softmax is uniform over all keys
    for b in range(B):
        vm = value[b].mean(0)
        x = np.concatenate([vm @ Wv[h] + bv[h] for h in range(H)])
        out[b, S - 1, :] = x @ Wo + bo
    return out


# revision 14
# speedup vs baseline: 1.1236x; 1.1236x over previous
"""Multi-head attention (strictly-upper-triangular mask variant) on 8 TRN2 cores.

Reference math (B=4, S=2048, D=512, H=8, A=64):
    q/k/v = per-head projections of query/key/value           [B,H,S,A]
    scores = q @ k^T / sqrt(A), lower triangle (incl diag) masked to -1e9
    out = concat_heads(softmax(scores) @ v) @ Wo + bo         [B,S,D]

Sharding: 8 cores = 4 batches x 2 interleaved q-tile sets.  Core c handles
batch b=c//2, q-tiles g = 2*i + (c%2) for i in 0..7 (128 rows each).  Every
core computes all 8 heads for its 1024 query rows; no collectives — the host
gather is a row-interleave concat.

Device-side design:
  * Q/K projections run in fp8e4 with DoubleRow perf mode (operands
    [K, 2, M]/[K, 2, N] contract two K=128 subtiles per pass — 2x the bf16
    column rate).  Score errors only perturb softmax weights (ratio-
    protected), so fp8 is safe here; weights are host-scaled x32 into
    e4m3's normal range and the combined 1/(32*32*sqrt(A)) is folded into
    the EXP activation's scale.  The value path (V projection, AV, output
    projection) stays bf16: rows attending to 1-2 keys emit essentially a
    raw v row, so e4m3's ~3% quantum there would exceed the error budget.
  * Scores are computed transposed (S^T[k,q]) in strips whose width tracks
    the causal boundary; strips pack into 6 exact 1536-wide PSUM bins so
    EXP runs as 6 big activations per head.  exp needs no max-subtraction
    (scores are O(1) bounded).
  * V is produced directly in natural [k,a] layout by swapping matmul
    roles (stationary = v^T data chunk, moving = Wv), eliminating PE
    transposes; a 64-wide ones block between the two heads' V blocks makes
    the AV matmul emit 64 replicated softmax denominators per head.
  * k-bias dropped (softmax shift invariance), v-bias folded into the
    output bias on host (softmax rows sum to 1), so K/V evictions are
    plain copies.
  * Triangular masks are 0/1 bf16 multiplies on the otherwise-idle GpSimd
    engine (last 128 columns of each key-chunk strip).
  * P^T strip storage ping-pongs between two SBUF tiles by head parity so
    head h+1's exp can overwrite while head h's AV still reads.

The single fully-masked query row (q = S-1, uniform attention in the
reference) comes back wrong from the device and is recomputed exactly on
the host during the gather.
"""

import numpy as np
import ml_dtypes

B, S, D, H, A = 4, 2048, 512, 8, 64
P = 128
NQ = 1024          # q rows per core
NQT = 8            # q tiles per core
NKC = 16           # k chunks
NPAIR = 4          # head pairs
BF = ml_dtypes.bfloat16
E4 = ml_dtypes.float8_e4m3

WSC = 32.0         # host scale on Wq/Wk (into e4m3 normal range)
EXP_SCALE = 1.0 / (WSC * WSC * 8.0)   # 2^-13: undo q,k weight scales + 1/sqrt(A)

# strip widths / offsets for the transposed-score layout
WKC = [P * (kc // 2 + 1) for kc in range(NKC)]
SOFF = np.concatenate([[0], np.cumsum(WKC)]).tolist()
PT_TOTAL = SOFF[-1]  # 9216
BINW = 1536
BIN_EDGE = list(range(0, PT_TOTAL, BINW)) + [PT_TOTAL]
NBINS = len(BIN_EDGE) - 1  # 6

_cache = {}


def _split512(a, b):
    """Split [a,b) at multiples of 512 (PSUM bank boundaries)."""
    out = []
    while a < b:
        nxt = min(b, (a // 512 + 1) * 512)
        out.append((a, nxt))
        a = nxt
    return out


def _build():
    if "nc" in _cache:
        return _cache["nc"]

    import concourse.bacc as bacc
    import concourse.mybir as mybir
    import concourse.tile as tile

    F32 = mybir.dt.float32
    BF16 = mybir.dt.bfloat16
    FP8 = mybir.dt.float8e4
    MULT = mybir.AluOpType.mult
    EXP = mybir.ActivationFunctionType.Exp
    DR = mybir.MatmulPerfMode.DoubleRow

    nc = bacc.Bacc("TRN2", target_bir_lowering=False, debug=False, num_devices=8)

    # batched inputs: few large DMAs, Q-projection-critical first
    inA_d = nc.dram_tensor("inA", [P, 2048 + 4 * NQ], FP8, kind="ExternalInput")
    inB_d = nc.dram_tensor("inB", [P, 2048 + 4 * S], FP8, kind="ExternalInput")
    inC_d = nc.dram_tensor("inC", [P, 2048 + 4 * S], BF16, kind="ExternalInput")
    inD_d = nc.dram_tensor("inD", [P, 2048 + 2 * P], BF16, kind="ExternalInput")
    bq_d = nc.dram_tensor("bq8", [P, 4], F32, kind="ExternalInput")
    bo_d = nc.dram_tensor("bo", [1, D], BF16, kind="ExternalInput")
    ones_d = nc.dram_tensor("ones1", [1, P], BF16, kind="ExternalInput")
    out_d = nc.dram_tensor("out", [NQ, D], F32, kind="ExternalOutput")

    # score-strip segments per exp bin: bin g -> [(kc, a0, a1), ...] global offsets
    bins = [[] for _ in range(NBINS)]
    for kc in range(NKC):
        for (a0, a1) in _split512(SOFF[kc], SOFF[kc] + WKC[kc]):
            g = a0 // BINW
            assert a1 <= BIN_EDGE[g + 1], (kc, a0, a1)
            bins[g].append((kc, a0, a1))

    with tile.TileContext(nc) as tc:
        with (
            tc.tile_pool(name="cst", bufs=1) as cst,
            tc.tile_pool(name="act", bufs=1) as act,
            tc.tile_pool(name="rcp", bufs=3) as rcp,
            tc.tile_pool(name="ost", bufs=3) as ost,
            tc.tile_pool(name="stg", bufs=2, space="PSUM") as stg,
            tc.tile_pool(name="avp", bufs=2, space="PSUM") as avp,
        ):
            # ---- constant loads: batched, in first-use order ----
            inA = cst.tile([P, 2048 + 4 * NQ], FP8, tag="inA")
            inB = cst.tile([P, 2048 + 4 * S], FP8, tag="inB")
            inC = cst.tile([P, 2048 + 4 * S], BF16, tag="inC")
            inD = cst.tile([P, 2048 + 2 * P], BF16, tag="inD")
            bq = cst.tile([P, 4], F32, tag="bq")
            bo = cst.tile([1, D], BF16, tag="bo")
            on1 = cst.tile([1, P], BF16, tag="on1")
            for t, d in [(inA, inA_d), (bq, bq_d), (inB, inB_d), (inC, inC_d),
                         (inD, inD_d), (bo, bo_d), (on1, ones_d)]:
                nc.sync.dma_start(t[:], d[:])
            wq, qT8 = inA[:, 0:2048], inA[:, 2048:]
            wk, kT = inB[:, 0:2048], inB[:, 2048:]
            wv, vT = inC[:, 0:2048], inC[:, 2048:]
            wo, mE, mO = inD[:, 0:2048], inD[:, 2048:2048 + P], inD[:, 2048 + P:]

            QT = [act.tile([P, NQ], BF16, tag=f"QT{p}", name=f"QT{p}") for p in range(NPAIR)]
            KT = [act.tile([P, S], BF16, tag=f"KT{p}", name=f"KT{p}") for p in range(NPAIR)]
            Vn = act.tile([P, NKC * 768], BF16, tag="Vn", name="Vn")
            # ping-pong by head parity: scores(h+1) writes its exp output
            # while attn_av(h) still reads head h's
            ptall = [act.tile([P, PT_TOTAL], BF16, tag=f"pt{i}", name=f"pt{i}")
                     for i in range(2)]
            XT = act.tile([P, 4 * NQ], BF16, tag="XT", name="XT")

            wq3 = wq.rearrange("k (b m) -> k b m", b=16)
            wk3 = wk.rearrange("k (b m) -> k b m", b=16)
            wv3 = wv.rearrange("k (c n) -> k c n", c=4)
            wo3 = wo.rearrange("k (c n) -> k c n", c=4)
            qT83 = qT8.rearrange("k (c n) -> k c n", c=4)
            kT3 = kT.rearrange("k (c n) -> k c n", c=4)
            vT3 = vT.rearrange("k (c n) -> k c n", c=4)
            Vn3 = Vn[:].rearrange("p (k c) -> p k c", c=768)
            # Vn block per k-chunk: 4x [V_h0(64) | ones(64) | V_h1(64)]
            Vn5 = Vn[:].rearrange("p (k q t f) -> p k q t f", q=4, t=3, f=64)
            XT3 = XT[:].rearrange("p (c n) -> p c n", c=4)

            # denominator ones blocks (written once)
            nc.gpsimd.memset(Vn5[:, :, :, 1, :], 1.0)

            # ---- projection pieces (fp8 DoubleRow, K=256; PSUM via avp pool) ----
            def proj_q(p, qh):
                ps = avp.tile([P, 512], F32, tag="av")
                for j in range(2):
                    nc.tensor.matmul(
                        ps[:], wq3[:, 4 * p + 2 * j:4 * p + 2 * j + 2, :],
                        qT83[:, 2 * j:2 * j + 2, 512 * qh:512 * (qh + 1)],
                        start=(j == 0), stop=(j == 1), perf_mode=DR)
                nc.vector.tensor_scalar_add(
                    QT[p][:, 512 * qh:512 * (qh + 1)], ps[:], bq[:, p:p + 1])

            def proj_k(p, sh):
                ps = avp.tile([P, 512], F32, tag="av")
                for j in range(2):
                    nc.tensor.matmul(
                        ps[:], wk3[:, 4 * p + 2 * j:4 * p + 2 * j + 2, :],
                        kT3[:, 2 * j:2 * j + 2, 512 * sh:512 * (sh + 1)],
                        start=(j == 0), stop=(j == 1), perf_mode=DR)
                nc.vector.tensor_copy(KT[p][:, 512 * sh:512 * (sh + 1)], ps[:])

            # V projection, natural [k,a] layout (stationary = data)
            def proj_v(kc):
                ps = avp.tile([P, 512], F32, tag="av")
                for ch in range(4):
                    nc.tensor.matmul(
                        ps[:], vT3[:, ch, P * kc:P * (kc + 1)],
                        wv3[:, ch, :],
                        start=(ch == 0), stop=(ch == 3))
                # psum cols = (pair, head-of-pair, a64) -> Vn V-blocks
                pv = ps[:].rearrange("p (q t f) -> p q t f", q=4, t=2, f=64)
                nc.vector.tensor_copy(Vn5[:, kc, :, 0, :], pv[:, :, 0, :])
                nc.vector.tensor_copy(Vn5[:, kc, :, 2, :], pv[:, :, 1, :])

            # ---- one score bin: matmuls + exp + masks-in-bin ----
            def scores_bin(h, g):
                p, hh = h // 2, h % 2
                pt = ptall[h % 2]
                hr = slice(64 * hh, 64 * hh + 64)
                e0, e1 = BIN_EDGE[g], BIN_EDGE[g + 1]
                st = stg.tile([P, BINW], F32, tag="big")
                for (kc, a0, a1) in bins[g]:
                    nc.tensor.matmul(
                        st[:, a0 - e0:a1 - e0],
                        KT[p][hr, P * kc:P * (kc + 1)],
                        QT[p][hr, a0 - SOFF[kc]:a1 - SOFF[kc]],
                        start=True, stop=True)
                nc.scalar.activation(
                    pt[:, e0:e1], st[:, 0:e1 - e0], EXP, scale=EXP_SCALE)
                # diagonal-tile masks: last 128 columns of each strip
                for kc in range(NKC):
                    dend = SOFF[kc] + WKC[kc]
                    if e0 < dend <= e1:
                        nc.gpsimd.tensor_tensor(
                            pt[:, dend - P:dend], pt[:, dend - P:dend],
                            mE if kc % 2 == 0 else mO, MULT)

            # ---- AV half (bf16) + normalize ----
            def av_half(h, b):
                p, hh = h // 2, h % 2
                pt = ptall[h % 2]
                hr = slice(64 * hh, 64 * hh + 64)
                po = 192 * p + 64 * hh
                orow = 0 if hh == 0 else 64
                drow = 64 - orow
                avb = avp.tile([P, 512], F32, tag="av")
                kc0 = 8 * b
                for kc in range(kc0, NKC):
                    w = min(WKC[kc], 512 * (b + 1)) - 512 * b
                    nc.tensor.matmul(
                        avb[:, 0:w],
                        Vn3[:, kc, po:po + 128],
                        pt[:, SOFF[kc] + 512 * b:SOFF[kc] + 512 * b + w],
                        start=(kc == kc0), stop=(kc == NKC - 1),
                        skip_group_check=True)
                rec = rcp.tile([64, 1024], F32, tag="rec")
                nc.vector.tensor_copy(rec[:, 0:512], avb[drow:drow + 64, :])
                nc.vector.reciprocal_approx_fast(rec[:, 512:1024], rec[:, 0:512])
                nc.vector.tensor_tensor(
                    XT3[hr, p, 512 * b:512 * (b + 1)],
                    avb[orow:orow + 64, :], rec[:, 512:1024], MULT)

            # ---- software-pipelined schedule ----
            # PE issue order interleaves each head's score bins with the
            # previous head's AV halves and the next pair's projections, so
            # the PE never drains while EXP catches up (keeps the p-state
            # clock ramped at 2.4GHz).
            for qh in range(2):
                proj_q(0, qh)
            for sh in range(4):
                proj_k(0, sh)

            # head 0: interleave V projection chunks between score bins
            scores_bin(0, 0)
            scores_bin(0, 1)
            vq = list(range(NKC))
            for g in range(2, NBINS):
                proj_v(vq.pop(0))
                proj_v(vq.pop(0))
                proj_v(vq.pop(0))
                scores_bin(0, g)
            while vq:
                proj_v(vq.pop(0))

            for h in range(1, H):
                p = h // 2
                xq = [lambda b=b: av_half(h - 1, b) for b in range(2)]
                if h % 2 == 1 and p + 1 < NPAIR:
                    xq += [lambda qh=qh: proj_q(p + 1, qh) for qh in range(2)]
                    xq += [lambda sh=sh: proj_k(p + 1, sh) for sh in range(4)]
                scores_bin(h, 0)
                scores_bin(h, 1)
                for g in range(2, NBINS):
                    if xq:
                        xq.pop(0)()
                    scores_bin(h, g)
                while xq:
                    xq.pop(0)()
            av_half(H - 1, 0)
            av_half(H - 1, 1)

            # ---- output projection (bf16) ----
            for i in range(NQT):
                po = avp.tile([P, D], F32, tag="av")
                for ch in range(4):
                    nc.tensor.matmul(
                        po[:], XT3[:, ch, P * i:P * (i + 1)],
                        wo3[:, ch, :],
                        start=(ch == 0), stop=False)
                nc.tensor.matmul(po[:], on1[0:1, :], bo[0:1, :],
                                 start=False, stop=True)
                ob = ost.tile([P, D], F32, tag="ob")
                nc.vector.tensor_copy(ob[:], po[:])
                nc.sync.dma_start(out_d[P * i:P * (i + 1), :], ob[:])

    nc.compile()
    _cache["nc"] = nc
    return nc


def _host_prep(query, key, value, Wq, bq, Wk, bk, Wv, bv, Wo, bo):
    """Build the 8 per-core input maps (all device-side layouts)."""
    def stack_chmin(W, scale, dt):
        # [H,D,A] -> [128, 16*128], block b=4p+ch: rows 128ch of [W_2p|W_2p+1]
        blocks = []
        for p in range(NPAIR):
            Wp = np.concatenate([W[2 * p], W[2 * p + 1]], axis=1) * scale
            for ch in range(4):
                blocks.append(Wp[P * ch:P * (ch + 1), :])
        return np.stack(blocks, 1).reshape(P, -1).astype(dt)

    def stack_pmin(W, dt):
        # [H,D,A] -> [128, 16*128], block b=4ch+p (moving operand layout)
        blocks = []
        for ch in range(4):
            for p in range(NPAIR):
                Wp = np.concatenate([W[2 * p], W[2 * p + 1]], axis=1)
                blocks.append(Wp[P * ch:P * (ch + 1), :])
        return np.stack(blocks, 1).reshape(P, -1).astype(dt)

    wq_h = stack_chmin(Wq, WSC, E4)
    wk_h = stack_chmin(Wk, WSC, E4)
    wv_h = stack_pmin(Wv, BF)
    wo_h = np.stack([Wo[P * ch:P * (ch + 1), :] for ch in range(4)], 1)
    wo_h = wo_h.reshape(P, -1).astype(BF)

    bq_h = np.stack(
        [np.concatenate([bq[2 * p], bq[2 * p + 1]]) * WSC for p in range(NPAIR)],
        1).astype(np.float32)
    # v-bias adds bv to each head's attention output (softmax rows sum to 1):
    # fold it, with bo, into the single output bias row
    bo_eff = bo + np.concatenate(list(bv)) @ Wo
    bo_h = bo_eff[None, :].astype(BF)
    ones_h = np.ones((1, P), BF)
    kl = np.arange(P)[:, None]
    ql = np.arange(P)[None, :]
    tril_strict = (kl > ql).astype(BF)

    def chunked_T(x, dt):
        # [S', D] -> [128, 4*S'] with col block ch = rows 128ch of x.T
        xT = np.ascontiguousarray(x.T)  # [512, S']
        return xT.reshape(4, P, -1).transpose(1, 0, 2).reshape(P, -1).astype(dt)

    in_maps = []
    for c in range(8):
        b, pair = c // 2, c % 2
        sel = np.concatenate(
            [np.arange(P * (2 * i + pair), P * (2 * i + pair) + P) for i in range(NQT)])
        mE_h = tril_strict if pair == 0 else np.zeros((P, P), BF)
        mO_h = np.ones((P, P), BF) if pair == 0 else tril_strict
        m = {
            "inA": np.concatenate([wq_h, chunked_T(query[b][sel], E4)], 1),
            "inB": np.concatenate([wk_h, chunked_T(key[b], E4)], 1),
            "inC": np.concatenate([wv_h, chunked_T(value[b], BF)], 1),
            "inD": np.concatenate([wo_h, mE_h, mO_h], 1),
            "bq8": bq_h, "bo": bo_h, "ones1": ones_h,
        }
        in_maps.append(m)
    return in_maps


def kernel(query, key, value, Wq, bq, Wk, bk, Wv, bv, Wo, bo):
    from concourse.bass_utils import run_bass_kernel_spmd

    args = [np.asarray(a, dtype=np.float32) for a in
            (query, key, value, Wq, bq, Wk, bk, Wv, bv, Wo, bo)]
    query, key, value, Wq, bq, Wk, bk, Wv, bv, Wo, bo = args

    nc = _build()
    in_maps = _host_prep(*args)
    res = run_bass_kernel_spmd(nc, in_maps, list(range(8)))

    out = np.empty((B, S, D), np.float32)
    for c in range(8):
        b, pair = c // 2, c % 2
        o = res.results[c]["out"]
        for i in range(NQT):
            g = 2 * i + pair
            out[b, P * g:P * (g + 1), :] = o[P * i:P * (i + 1), :]

    # q = S-1 attends to nothing -> reference softmax is uniform over all keys
    for b in range(B):
        vm = value[b].mean(0)
        x = np.concatenate([vm @ Wv[h] + bv[h] for h in range(H)])
        out[b, S - 1, :] = x @ Wo + bo
    return out


# revision 16
# speedup vs baseline: 1.1496x; 1.0231x over previous
"""Multi-head attention (strictly-upper-triangular mask variant) on 8 TRN2 cores.

Reference math (B=4, S=2048, D=512, H=8, A=64):
    q/k/v = per-head projections of query/key/value           [B,H,S,A]
    scores = q @ k^T / sqrt(A), lower triangle (incl diag) masked to -1e9
    out = concat_heads(softmax(scores) @ v) @ Wo + bo         [B,S,D]

Sharding: 8 cores = 4 batches x 2 interleaved q-tile sets.  Core c handles
batch b=c//2, q-tiles g = 2*i + (c%2) for i in 0..7 (128 rows each).  Every
core computes all 8 heads for its 1024 query rows; no collectives — the host
gather is a row-interleave concat.

Device-side design:
  * Q/K projections run in fp8e4 with DoubleRow perf mode (operands
    [K, 2, M]/[K, 2, N] contract two K=128 subtiles per pass — 2x the bf16
    column rate).  Score errors only perturb softmax weights (ratio-
    protected), so fp8 is safe here; weights are host-scaled x32 into
    e4m3's normal range and the combined 1/(32*32*sqrt(A)) is folded into
    the EXP activation's scale.  The value path (V projection, AV, output
    projection) stays bf16: rows attending to 1-2 keys emit essentially a
    raw v row, so e4m3's ~3% quantum there would exceed the error budget.
  * Scores are computed transposed (S^T[k,q]) in strips whose width tracks
    the causal boundary; strips pack into 6 exact 1536-wide PSUM bins so
    EXP runs as 6 big activations per head.  exp needs no max-subtraction
    (scores are O(1) bounded).
  * V is produced directly in natural [k,a] layout by swapping matmul
    roles (stationary = v^T data chunk, moving = Wv), eliminating PE
    transposes; a 64-wide ones block between the two heads' V blocks makes
    the AV matmul emit 64 replicated softmax denominators per head.
  * k-bias dropped (softmax shift invariance), v-bias folded into the
    output bias on host (softmax rows sum to 1), so K/V evictions are
    plain copies.
  * Triangular masks are 0/1 bf16 multiplies on the otherwise-idle GpSimd
    engine (last 128 columns of each key-chunk strip).
  * P^T strip storage ping-pongs between two SBUF tiles by head parity so
    head h+1's exp can overwrite while head h's AV still reads.

The single fully-masked query row (q = S-1, uniform attention in the
reference) comes back wrong from the device and is recomputed exactly on
the host during the gather.
"""

import numpy as np
import ml_dtypes

B, S, D, H, A = 4, 2048, 512, 8, 64
P = 128
NQ = 1024          # q rows per core
NQT = 8            # q tiles per core
NKC = 16           # k chunks
NPAIR = 4          # head pairs
BF = ml_dtypes.bfloat16
E4 = ml_dtypes.float8_e4m3

WSC = 32.0         # host scale on Wq/Wk (into e4m3 normal range)
EXP_SCALE = 1.0 / (WSC * WSC * 8.0)   # 2^-13: undo q,k weight scales + 1/sqrt(A)

# strip widths / offsets for the transposed-score layout
WKC = [P * (kc // 2 + 1) for kc in range(NKC)]
SOFF = np.concatenate([[0], np.cumsum(WKC)]).tolist()
PT_TOTAL = SOFF[-1]  # 9216
BINW = 1536
BIN_EDGE = list(range(0, PT_TOTAL, BINW)) + [PT_TOTAL]
NBINS = len(BIN_EDGE) - 1  # 6

_cache = {}


def _split512(a, b):
    """Split [a,b) at multiples of 512 (PSUM bank boundaries)."""
    out = []
    while a < b:
        nxt = min(b, (a // 512 + 1) * 512)
        out.append((a, nxt))
        a = nxt
    return out


def _build():
    if "nc" in _cache:
        return _cache["nc"]

    import concourse.bacc as bacc
    import concourse.mybir as mybir
    import concourse.tile as tile

    F32 = mybir.dt.float32
    BF16 = mybir.dt.bfloat16
    FP8 = mybir.dt.float8e4
    MULT = mybir.AluOpType.mult
    EXP = mybir.ActivationFunctionType.Exp
    DR = mybir.MatmulPerfMode.DoubleRow

    nc = bacc.Bacc("TRN2", target_bir_lowering=False, debug=False, num_devices=8)

    # batched inputs: few large DMAs, Q-projection-critical first
    inA_d = nc.dram_tensor("inA", [P, 2048 + 4 * NQ], FP8, kind="ExternalInput")
    inB_d = nc.dram_tensor("inB", [P, 2048 + 4 * S], FP8, kind="ExternalInput")
    inC_d = nc.dram_tensor("inC", [P, 2048 + 4 * S], BF16, kind="ExternalInput")
    inD_d = nc.dram_tensor("inD", [P, 2048 + 2 * P], BF16, kind="ExternalInput")
    bq_d = nc.dram_tensor("bq8", [P, 4], F32, kind="ExternalInput")
    bo_d = nc.dram_tensor("bo", [1, D], BF16, kind="ExternalInput")
    ones_d = nc.dram_tensor("ones1", [1, P], BF16, kind="ExternalInput")
    out_d = nc.dram_tensor("out", [NQ, D], F32, kind="ExternalOutput")

    # score-strip segments per exp bin: bin g -> [(kc, a0, a1), ...] global offsets
    bins = [[] for _ in range(NBINS)]
    for kc in range(NKC):
        for (a0, a1) in _split512(SOFF[kc], SOFF[kc] + WKC[kc]):
            g = a0 // BINW
            assert a1 <= BIN_EDGE[g + 1], (kc, a0, a1)
            bins[g].append((kc, a0, a1))

    with tile.TileContext(nc) as tc:
        with (
            tc.tile_pool(name="cst", bufs=1) as cst,
            tc.tile_pool(name="act", bufs=1) as act,
            tc.tile_pool(name="rcp", bufs=3) as rcp,
            tc.tile_pool(name="ost", bufs=3) as ost,
            tc.tile_pool(name="stg", bufs=2, space="PSUM") as stg,
            tc.tile_pool(name="avp", bufs=2, space="PSUM") as avp,
        ):
            # ---- constant loads: batched, in first-use order ----
            inA = cst.tile([P, 2048 + 4 * NQ], FP8, tag="inA")
            inB = cst.tile([P, 2048 + 4 * S], FP8, tag="inB")
            inC = cst.tile([P, 2048 + 4 * S], BF16, tag="inC")
            inD = cst.tile([P, 2048 + 2 * P], BF16, tag="inD")
            bq = cst.tile([P, 4], F32, tag="bq")
            bo = cst.tile([1, D], BF16, tag="bo")
            on1 = cst.tile([1, P], BF16, tag="on1")
            # inB split so K-projection's first seq-half lands sooner
            # (kT is [128, (ch, S)]: slice by sequence position, not prefix)
            nc.sync.dma_start(inA[:], inA_d[:])
            nc.sync.dma_start(bq[:], bq_d[:])
            nc.sync.dma_start(inB[:, 0:2048], inB_d[:, 0:2048])
            kt_sb = inB[:, 2048:].rearrange("k (c n) -> k c n", c=4)
            kt_dr = inB_d[:, 2048:].rearrange("k (c n) -> k c n", c=4)
            nc.sync.dma_start(kt_sb[:, :, 0:1024], kt_dr[:, :, 0:1024])
            nc.sync.dma_start(kt_sb[:, :, 1024:2048], kt_dr[:, :, 1024:2048])
            for t, d in [(inC, inC_d), (inD, inD_d), (bo, bo_d), (on1, ones_d)]:
                nc.sync.dma_start(t[:], d[:])
            wq, qT8 = inA[:, 0:2048], inA[:, 2048:]
            wk, kT = inB[:, 0:2048], inB[:, 2048:]
            wv, vT = inC[:, 0:2048], inC[:, 2048:]
            wo, mE, mO = inD[:, 0:2048], inD[:, 2048:2048 + P], inD[:, 2048 + P:]

            QT = [act.tile([P, NQ], BF16, tag=f"QT{p}", name=f"QT{p}") for p in range(NPAIR)]
            KT = [act.tile([P, S], BF16, tag=f"KT{p}", name=f"KT{p}") for p in range(NPAIR)]
            Vn = act.tile([P, NKC * 768], BF16, tag="Vn", name="Vn")
            # ping-pong by head parity: scores(h+1) writes its exp output
            # while attn_av(h) still reads head h's
            ptall = [act.tile([P, PT_TOTAL], BF16, tag=f"pt{i}", name=f"pt{i}")
                     for i in range(2)]
            XT = act.tile([P, 4 * NQ], BF16, tag="XT", name="XT")

            wq3 = wq.rearrange("k (b m) -> k b m", b=16)
            wk3 = wk.rearrange("k (b m) -> k b m", b=16)
            wv3 = wv.rearrange("k (c n) -> k c n", c=4)
            wo3 = wo.rearrange("k (c n) -> k c n", c=4)
            qT83 = qT8.rearrange("k (c n) -> k c n", c=4)
            kT3 = kT.rearrange("k (c n) -> k c n", c=4)
            vT3 = vT.rearrange("k (c n) -> k c n", c=4)
            Vn3 = Vn[:].rearrange("p (k c) -> p k c", c=768)
            # Vn block per k-chunk: 4x [V_h0(64) | ones(64) | V_h1(64)]
            Vn5 = Vn[:].rearrange("p (k q t f) -> p k q t f", q=4, t=3, f=64)
            XT3 = XT[:].rearrange("p (c n) -> p c n", c=4)

            # denominator ones blocks (written once)
            nc.gpsimd.memset(Vn5[:, :, :, 1, :], 1.0)

            # ---- projection pieces (fp8 DoubleRow, K=256; PSUM via avp pool) ----
            def proj_q(p, qh):
                ps = avp.tile([P, 512], F32, tag="av")
                for j in range(2):
                    nc.tensor.matmul(
                        ps[:], wq3[:, 4 * p + 2 * j:4 * p + 2 * j + 2, :],
                        qT83[:, 2 * j:2 * j + 2, 512 * qh:512 * (qh + 1)],
                        start=(j == 0), stop=(j == 1), perf_mode=DR)
                nc.vector.tensor_scalar_add(
                    QT[p][:, 512 * qh:512 * (qh + 1)], ps[:], bq[:, p:p + 1])

            def proj_k(p, sh):
                ps = avp.tile([P, 512], F32, tag="av")
                for j in range(2):
                    nc.tensor.matmul(
                        ps[:], wk3[:, 4 * p + 2 * j:4 * p + 2 * j + 2, :],
                        kT3[:, 2 * j:2 * j + 2, 512 * sh:512 * (sh + 1)],
                        start=(j == 0), stop=(j == 1), perf_mode=DR)
                nc.vector.tensor_copy(KT[p][:, 512 * sh:512 * (sh + 1)], ps[:])

            # V projection, natural [k,a] layout (stationary = data)
            def proj_v(kc):
                ps = avp.tile([P, 512], F32, tag="av")
                for ch in range(4):
                    nc.tensor.matmul(
                        ps[:], vT3[:, ch, P * kc:P * (kc + 1)],
                        wv3[:, ch, :],
                        start=(ch == 0), stop=(ch == 3))
                # psum cols = (pair, head-of-pair, a64) -> Vn V-blocks
                pv = ps[:].rearrange("p (q t f) -> p q t f", q=4, t=2, f=64)
                nc.vector.tensor_copy(Vn5[:, kc, :, 0, :], pv[:, :, 0, :])
                nc.vector.tensor_copy(Vn5[:, kc, :, 2, :], pv[:, :, 1, :])

            # ---- one score bin: matmuls + exp + masks-in-bin ----
            def scores_bin(h, g):
                p, hh = h // 2, h % 2
                pt = ptall[h % 2]
                hr = slice(64 * hh, 64 * hh + 64)
                e0, e1 = BIN_EDGE[g], BIN_EDGE[g + 1]
                st = stg.tile([P, BINW], F32, tag="big")
                for (kc, a0, a1) in bins[g]:
                    nc.tensor.matmul(
                        st[:, a0 - e0:a1 - e0],
                        KT[p][hr, P * kc:P * (kc + 1)],
                        QT[p][hr, a0 - SOFF[kc]:a1 - SOFF[kc]],
                        start=True, stop=True)
                nc.scalar.activation(
                    pt[:, e0:e1], st[:, 0:e1 - e0], EXP, scale=EXP_SCALE)
                # diagonal-tile masks: last 128 columns of each strip
                for kc in range(NKC):
                    dend = SOFF[kc] + WKC[kc]
                    if e0 < dend <= e1:
                        nc.gpsimd.tensor_tensor(
                            pt[:, dend - P:dend], pt[:, dend - P:dend],
                            mE if kc % 2 == 0 else mO, MULT)

            # ---- AV half (bf16) + normalize ----
            def av_half(h, b):
                p, hh = h // 2, h % 2
                pt = ptall[h % 2]
                hr = slice(64 * hh, 64 * hh + 64)
                po = 192 * p + 64 * hh
                orow = 0 if hh == 0 else 64
                drow = 64 - orow
                avb = avp.tile([P, 512], F32, tag="av")
                kc0 = 8 * b
                for kc in range(kc0, NKC):
                    w = min(WKC[kc], 512 * (b + 1)) - 512 * b
                    nc.tensor.matmul(
                        avb[:, 0:w],
                        Vn3[:, kc, po:po + 128],
                        pt[:, SOFF[kc] + 512 * b:SOFF[kc] + 512 * b + w],
                        start=(kc == kc0), stop=(kc == NKC - 1),
                        skip_group_check=True)
                rec = rcp.tile([64, 1024], F32, tag="rec")
                nc.vector.tensor_copy(rec[:, 0:512], avb[drow:drow + 64, :])
                nc.vector.reciprocal_approx_fast(rec[:, 512:1024], rec[:, 0:512])
                nc.vector.tensor_tensor(
                    XT3[hr, p, 512 * b:512 * (b + 1)],
                    avb[orow:orow + 64, :], rec[:, 512:1024], MULT)

            # ---- software-pipelined schedule ----
            # PE issue order interleaves each head's score bins with the
            # previous head's AV halves and the next pair's projections, so
            # the PE never drains while EXP catches up (keeps the p-state
            # clock ramped at 2.4GHz).
            for qh in range(2):
                proj_q(0, qh)
            for sh in range(4):
                proj_k(0, sh)

            # head 0: interleave V projection chunks between score bins
            scores_bin(0, 0)
            scores_bin(0, 1)
            vq = list(range(NKC))
            for g in range(2, NBINS):
                proj_v(vq.pop(0))
                proj_v(vq.pop(0))
                proj_v(vq.pop(0))
                scores_bin(0, g)
            while vq:
                proj_v(vq.pop(0))

            for h in range(1, H):
                p = h // 2
                xq = [lambda b=b: av_half(h - 1, b) for b in range(2)]
                if h % 2 == 1 and p + 1 < NPAIR:
                    xq += [lambda qh=qh: proj_q(p + 1, qh) for qh in range(2)]
                    xq += [lambda sh=sh: proj_k(p + 1, sh) for sh in range(4)]
                scores_bin(h, 0)
                scores_bin(h, 1)
                for g in range(2, NBINS):
                    if xq:
                        xq.pop(0)()
                    scores_bin(h, g)
                while xq:
                    xq.pop(0)()
            av_half(H - 1, 0)
            av_half(H - 1, 1)

            # ---- output projection (bf16) ----
            for i in range(NQT):
                # stg pool: avp's buffers are still pinned by the last
                # head's AV tiles awaiting their DVE normalize chains
                po = stg.tile([P, D], F32, tag="big", padded_shape=[P, BINW])
                for ch in range(4):
                    nc.tensor.matmul(
                        po[:], XT3[:, ch, P * i:P * (i + 1)],
                        wo3[:, ch, :],
                        start=(ch == 0), stop=False)
                nc.tensor.matmul(po[:], on1[0:1, :], bo[0:1, :],
                                 start=False, stop=True)
                ob = ost.tile([P, D], F32, tag="ob")
                nc.vector.tensor_copy(ob[:], po[:])
                nc.sync.dma_start(out_d[P * i:P * (i + 1), :], ob[:])

    nc.compile()
    _cache["nc"] = nc
    return nc


def _host_prep(query, key, value, Wq, bq, Wk, bk, Wv, bv, Wo, bo):
    """Build the 8 per-core input maps (all device-side layouts)."""
    def stack_chmin(W, scale, dt):
        # [H,D,A] -> [128, 16*128], block b=4p+ch: rows 128ch of [W_2p|W_2p+1]
        blocks = []
        for p in range(NPAIR):
            Wp = np.concatenate([W[2 * p], W[2 * p + 1]], axis=1) * scale
            for ch in range(4):
                blocks.append(Wp[P * ch:P * (ch + 1), :])
        return np.stack(blocks, 1).reshape(P, -1).astype(dt)

    def stack_pmin(W, dt):
        # [H,D,A] -> [128, 16*128], block b=4ch+p (moving operand layout)
        blocks = []
        for ch in range(4):
            for p in range(NPAIR):
                Wp = np.concatenate([W[2 * p], W[2 * p + 1]], axis=1)
                blocks.append(Wp[P * ch:P * (ch + 1), :])
        return np.stack(blocks, 1).reshape(P, -1).astype(dt)

    wq_h = stack_chmin(Wq, WSC, E4)
    wk_h = stack_chmin(Wk, WSC, E4)
    wv_h = stack_pmin(Wv, BF)
    wo_h = np.stack([Wo[P * ch:P * (ch + 1), :] for ch in range(4)], 1)
    wo_h = wo_h.reshape(P, -1).astype(BF)

    bq_h = np.stack(
        [np.concatenate([bq[2 * p], bq[2 * p + 1]]) * WSC for p in range(NPAIR)],
        1).astype(np.float32)
    # v-bias adds bv to each head's attention output (softmax rows sum to 1):
    # fold it, with bo, into the single output bias row
    bo_eff = bo + np.concatenate(list(bv)) @ Wo
    bo_h = bo_eff[None, :].astype(BF)
    ones_h = np.ones((1, P), BF)
    kl = np.arange(P)[:, None]
    ql = np.arange(P)[None, :]
    tril_strict = (kl > ql).astype(BF)

    def chunked_T(x, dt):
        # [S', D] -> [128, 4*S'] with col block ch = rows 128ch of x.T
        xT = np.ascontiguousarray(x.T)  # [512, S']
        return xT.reshape(4, P, -1).transpose(1, 0, 2).reshape(P, -1).astype(dt)

    in_maps = []
    for c in range(8):
        b, pair = c // 2, c % 2
        sel = np.concatenate(
            [np.arange(P * (2 * i + pair), P * (2 * i + pair) + P) for i in range(NQT)])
        mE_h = tril_strict if pair == 0 else np.zeros((P, P), BF)
        mO_h = np.ones((P, P), BF) if pair == 0 else tril_strict
        m = {
            "inA": np.concatenate([wq_h, chunked_T(query[b][sel], E4)], 1),
            "inB": np.concatenate([wk_h, chunked_T(key[b], E4)], 1),
            "inC": np.concatenate([wv_h, chunked_T(value[b], BF)], 1),
            "inD": np.concatenate([wo_h, mE_h, mO_h], 1),
            "bq8": bq_h, "bo": bo_h, "ones1": ones_h,
        }
        in_maps.append(m)
    return in_maps


def kernel(query, key, value, Wq, bq, Wk, bk, Wv, bv, Wo, bo):
    from concourse.bass_utils import run_bass_kernel_spmd

    args = [np.asarray(a, dtype=np.float32) for a in
            (query, key, value, Wq, bq, Wk, bk, Wv, bv, Wo, bo)]
    query, key, value, Wq, bq, Wk, bk, Wv, bv, Wo, bo = args

    nc = _build()
    in_maps = _host_prep(*args)
    res = run_bass_kernel_spmd(nc, in_maps, list(range(8)))

    out = np.empty((B, S, D), np.float32)
    for c in range(8):
        b, pair = c // 2, c % 2
        o = res.results[c]["out"]
        for i in range(NQT):
            g = 2 * i + pair
            out[b, P * g:P * (g + 1), :] = o[P * i:P * (i + 1), :]

    # q = S-1 attends to nothing -> reference softmax is uniform over all keys
    for b in range(B):
        vm = value[b].mean(0)
        x = np.concatenate([vm @ Wv[h] + bv[h] for h in range(H)])
        out[b, S - 1, :] = x @ Wo + bo
    return out


# revision 17
# speedup vs baseline: 1.1595x; 1.0087x over previous
"""Multi-head attention (strictly-upper-triangular mask variant) on 8 TRN2 cores.

Reference math (B=4, S=2048, D=512, H=8, A=64):
    q/k/v = per-head projections of query/key/value           [B,H,S,A]
    scores = q @ k^T / sqrt(A), lower triangle (incl diag) masked to -1e9
    out = concat_heads(softmax(scores) @ v) @ Wo + bo         [B,S,D]

Sharding: 8 cores = 4 batches x 2 interleaved q-tile sets.  Core c handles
batch b=c//2, q-tiles g = 2*i + (c%2) for i in 0..7 (128 rows each).  Every
core computes all 8 heads for its 1024 query rows; no collectives — the host
gather is a row-interleave concat.

Device-side design:
  * Q/K projections run in fp8e4 with DoubleRow perf mode (operands
    [K, 2, M]/[K, 2, N] contract two K=128 subtiles per pass — 2x the bf16
    column rate).  Score errors only perturb softmax weights (ratio-
    protected), so fp8 is safe here; weights are host-scaled x32 into
    e4m3's normal range and the combined 1/(32*32*sqrt(A)) is folded into
    the EXP activation's scale.  The value path (V projection, AV, output
    projection) stays bf16: rows attending to 1-2 keys emit essentially a
    raw v row, so e4m3's ~3% quantum there would exceed the error budget.
  * Scores are computed transposed (S^T[k,q]) in strips whose width tracks
    the causal boundary; strips pack into 6 exact 1536-wide PSUM bins so
    EXP runs as 6 big activations per head.  exp needs no max-subtraction
    (scores are O(1) bounded).
  * V is produced directly in natural [k,a] layout by swapping matmul
    roles (stationary = v^T data chunk, moving = Wv), eliminating PE
    transposes; a 64-wide ones block between the two heads' V blocks makes
    the AV matmul emit 64 replicated softmax denominators per head.
  * k-bias dropped (softmax shift invariance), v-bias folded into the
    output bias on host (softmax rows sum to 1), so K/V evictions are
    plain copies.
  * Triangular masks are 0/1 bf16 multiplies on the otherwise-idle GpSimd
    engine (last 128 columns of each key-chunk strip).
  * P^T strip storage ping-pongs between two SBUF tiles by head parity so
    head h+1's exp can overwrite while head h's AV still reads.

The single fully-masked query row (q = S-1, uniform attention in the
reference) comes back wrong from the device and is recomputed exactly on
the host during the gather.
"""

import numpy as np
import ml_dtypes

B, S, D, H, A = 4, 2048, 512, 8, 64
P = 128
NQ = 1024          # q rows per core
NQT = 8            # q tiles per core
NKC = 16           # k chunks
NPAIR = 4          # head pairs
BF = ml_dtypes.bfloat16
E4 = ml_dtypes.float8_e4m3

WSC = 32.0         # host scale on Wq/Wk (into e4m3 normal range)
EXP_SCALE = 1.0 / (WSC * WSC * 8.0)   # 2^-13: undo q,k weight scales + 1/sqrt(A)

# strip widths / offsets for the transposed-score layout
WKC = [P * (kc // 2 + 1) for kc in range(NKC)]
SOFF = np.concatenate([[0], np.cumsum(WKC)]).tolist()
PT_TOTAL = SOFF[-1]  # 9216
BINW = 1536
BIN_EDGE = list(range(0, PT_TOTAL, BINW)) + [PT_TOTAL]
NBINS = len(BIN_EDGE) - 1  # 6

_cache = {}


def _split512(a, b):
    """Split [a,b) at multiples of 512 (PSUM bank boundaries)."""
    out = []
    while a < b:
        nxt = min(b, (a // 512 + 1) * 512)
        out.append((a, nxt))
        a = nxt
    return out


def _build():
    if "nc" in _cache:
        return _cache["nc"]

    import concourse.bacc as bacc
    import concourse.mybir as mybir
    import concourse.tile as tile

    F32 = mybir.dt.float32
    BF16 = mybir.dt.bfloat16
    FP8 = mybir.dt.float8e4
    MULT = mybir.AluOpType.mult
    EXP = mybir.ActivationFunctionType.Exp
    DR = mybir.MatmulPerfMode.DoubleRow

    nc = bacc.Bacc("TRN2", target_bir_lowering=False, debug=False, num_devices=8)

    # batched inputs: few large DMAs, Q-projection-critical first
    inA_d = nc.dram_tensor("inA", [P, 2048 + 4 * NQ], FP8, kind="ExternalInput")
    inB_d = nc.dram_tensor("inB", [P, 2048 + 4 * S], FP8, kind="ExternalInput")
    inC_d = nc.dram_tensor("inC", [P, 2048 + 4 * S], BF16, kind="ExternalInput")
    inD_d = nc.dram_tensor("inD", [P, 2048 + 2 * P], BF16, kind="ExternalInput")
    bq_d = nc.dram_tensor("bq8", [P, 4], F32, kind="ExternalInput")
    bo_d = nc.dram_tensor("bo", [1, D], BF16, kind="ExternalInput")
    ones_d = nc.dram_tensor("ones1", [1, P], BF16, kind="ExternalInput")
    out_d = nc.dram_tensor("out", [NQ, D], F32, kind="ExternalOutput")

    # score-strip segments per exp bin: bin g -> [(kc, a0, a1), ...] global offsets
    bins = [[] for _ in range(NBINS)]
    for kc in range(NKC):
        for (a0, a1) in _split512(SOFF[kc], SOFF[kc] + WKC[kc]):
            g = a0 // BINW
            assert a1 <= BIN_EDGE[g + 1], (kc, a0, a1)
            bins[g].append((kc, a0, a1))

    with tile.TileContext(nc) as tc:
        with (
            tc.tile_pool(name="cst", bufs=1) as cst,
            tc.tile_pool(name="act", bufs=1) as act,
            tc.tile_pool(name="rcp", bufs=3) as rcp,
            tc.tile_pool(name="ost", bufs=3) as ost,
            tc.tile_pool(name="stg", bufs=2, space="PSUM") as stg,
            tc.tile_pool(name="avp", bufs=2, space="PSUM") as avp,
        ):
            # ---- constant loads: batched, in first-use order ----
            inA = cst.tile([P, 2048 + 4 * NQ], FP8, tag="inA")
            inB = cst.tile([P, 2048 + 4 * S], FP8, tag="inB")
            inC = cst.tile([P, 2048 + 4 * S], BF16, tag="inC")
            inD = cst.tile([P, 2048 + 2 * P], BF16, tag="inD")
            bq = cst.tile([P, 4], F32, tag="bq")
            bo = cst.tile([1, D], BF16, tag="bo")
            on1 = cst.tile([1, P], BF16, tag="on1")
            # inB split so K-projection's first seq-half lands sooner
            # (kT is [128, (ch, S)]: slice by sequence position, not prefix)
            nc.sync.dma_start(inA[:], inA_d[:])
            nc.sync.dma_start(bq[:], bq_d[:])
            nc.sync.dma_start(inB[:, 0:2048], inB_d[:, 0:2048])
            kt_sb = inB[:, 2048:].rearrange("k (c n) -> k c n", c=4)
            kt_dr = inB_d[:, 2048:].rearrange("k (c n) -> k c n", c=4)
            nc.sync.dma_start(kt_sb[:, :, 0:1024], kt_dr[:, :, 0:1024])
            nc.sync.dma_start(kt_sb[:, :, 1024:2048], kt_dr[:, :, 1024:2048])
            for t, d in [(inC, inC_d), (inD, inD_d), (bo, bo_d), (on1, ones_d)]:
                nc.sync.dma_start(t[:], d[:])
            wq, qT8 = inA[:, 0:2048], inA[:, 2048:]
            wk, kT = inB[:, 0:2048], inB[:, 2048:]
            wv, vT = inC[:, 0:2048], inC[:, 2048:]
            wo, mE, mO = inD[:, 0:2048], inD[:, 2048:2048 + P], inD[:, 2048 + P:]

            QT = [act.tile([P, NQ], BF16, tag=f"QT{p}", name=f"QT{p}") for p in range(NPAIR)]
            KT = [act.tile([P, S], BF16, tag=f"KT{p}", name=f"KT{p}") for p in range(NPAIR)]
            Vn = act.tile([P, NKC * 768], BF16, tag="Vn", name="Vn")
            # ping-pong by head parity: scores(h+1) writes its exp output
            # while attn_av(h) still reads head h's
            ptall = [act.tile([P, PT_TOTAL], BF16, tag=f"pt{i}", name=f"pt{i}")
                     for i in range(2)]
            XT = act.tile([P, 4 * NQ], BF16, tag="XT", name="XT")

            wq3 = wq.rearrange("k (b m) -> k b m", b=16)
            wk3 = wk.rearrange("k (b m) -> k b m", b=16)
            wv3 = wv.rearrange("k (c n) -> k c n", c=4)
            wo3 = wo.rearrange("k (c n) -> k c n", c=4)
            qT83 = qT8.rearrange("k (c n) -> k c n", c=4)
            kT3 = kT.rearrange("k (c n) -> k c n", c=4)
            vT3 = vT.rearrange("k (c n) -> k c n", c=4)
            Vn3 = Vn[:].rearrange("p (k c) -> p k c", c=768)
            # Vn block per k-chunk: 4x [V_h0(64) | ones(64) | V_h1(64)]
            Vn5 = Vn[:].rearrange("p (k q t f) -> p k q t f", q=4, t=3, f=64)
            XT3 = XT[:].rearrange("p (c n) -> p c n", c=4)

            # denominator ones blocks (written once)
            nc.gpsimd.memset(Vn5[:, :, :, 1, :], 1.0)

            # ---- projection pieces (fp8 DoubleRow, K=256; PSUM via avp pool) ----
            def proj_q(p, qh):
                ps = avp.tile([P, 512], F32, tag="av")
                for j in range(2):
                    nc.tensor.matmul(
                        ps[:], wq3[:, 4 * p + 2 * j:4 * p + 2 * j + 2, :],
                        qT83[:, 2 * j:2 * j + 2, 512 * qh:512 * (qh + 1)],
                        start=(j == 0), stop=(j == 1), perf_mode=DR)
                nc.vector.tensor_scalar_add(
                    QT[p][:, 512 * qh:512 * (qh + 1)], ps[:], bq[:, p:p + 1])

            def proj_k(p, sh):
                ps = avp.tile([P, 512], F32, tag="av")
                for j in range(2):
                    nc.tensor.matmul(
                        ps[:], wk3[:, 4 * p + 2 * j:4 * p + 2 * j + 2, :],
                        kT3[:, 2 * j:2 * j + 2, 512 * sh:512 * (sh + 1)],
                        start=(j == 0), stop=(j == 1), perf_mode=DR)
                nc.vector.tensor_copy(KT[p][:, 512 * sh:512 * (sh + 1)], ps[:])

            # V projection, natural [k,a] layout (stationary = data)
            def proj_v(kc):
                ps = avp.tile([P, 512], F32, tag="av")
                for ch in range(4):
                    nc.tensor.matmul(
                        ps[:], vT3[:, ch, P * kc:P * (kc + 1)],
                        wv3[:, ch, :],
                        start=(ch == 0), stop=(ch == 3))
                # psum cols = (pair, head-of-pair, a64) -> Vn V-blocks
                pv = ps[:].rearrange("p (q t f) -> p q t f", q=4, t=2, f=64)
                nc.vector.tensor_copy(Vn5[:, kc, :, 0, :], pv[:, :, 0, :])
                nc.vector.tensor_copy(Vn5[:, kc, :, 2, :], pv[:, :, 1, :])

            # ---- one score bin: matmuls + exp + masks-in-bin ----
            def scores_bin(h, g):
                p, hh = h // 2, h % 2
                pt = ptall[h % 2]
                hr = slice(64 * hh, 64 * hh + 64)
                e0, e1 = BIN_EDGE[g], BIN_EDGE[g + 1]
                st = stg.tile([P, BINW], F32, tag="big")
                for (kc, a0, a1) in bins[g]:
                    nc.tensor.matmul(
                        st[:, a0 - e0:a1 - e0],
                        KT[p][hr, P * kc:P * (kc + 1)],
                        QT[p][hr, a0 - SOFF[kc]:a1 - SOFF[kc]],
                        start=True, stop=True)
                nc.scalar.activation(
                    pt[:, e0:e1], st[:, 0:e1 - e0], EXP, scale=EXP_SCALE)
                # diagonal-tile masks: last 128 columns of each strip
                for kc in range(NKC):
                    dend = SOFF[kc] + WKC[kc]
                    if e0 < dend <= e1:
                        nc.gpsimd.tensor_tensor(
                            pt[:, dend - P:dend], pt[:, dend - P:dend],
                            mE if kc % 2 == 0 else mO, MULT)

            # ---- AV half (bf16) + normalize ----
            def av_half(h, b):
                p, hh = h // 2, h % 2
                pt = ptall[h % 2]
                hr = slice(64 * hh, 64 * hh + 64)
                po = 192 * p + 64 * hh
                orow = 0 if hh == 0 else 64
                drow = 64 - orow
                avb = avp.tile([P, 512], F32, tag="av")
                kc0 = 8 * b
                for kc in range(kc0, NKC):
                    w = min(WKC[kc], 512 * (b + 1)) - 512 * b
                    nc.tensor.matmul(
                        avb[:, 0:w],
                        Vn3[:, kc, po:po + 128],
                        pt[:, SOFF[kc] + 512 * b:SOFF[kc] + 512 * b + w],
                        start=(kc == kc0), stop=(kc == NKC - 1),
                        skip_group_check=True)
                rec = rcp.tile([64, 1024], F32, tag="rec")
                nc.vector.tensor_copy(rec[:, 0:512], avb[drow:drow + 64, :])
                nc.vector.reciprocal_approx_fast(rec[:, 512:1024], rec[:, 0:512])
                nc.vector.tensor_tensor(
                    XT3[hr, p, 512 * b:512 * (b + 1)],
                    avb[orow:orow + 64, :], rec[:, 512:1024], MULT)

            # ---- software-pipelined schedule ----
            # PE issue order interleaves each head's score bins with the
            # previous head's AV halves and the next pair's projections, so
            # the PE never drains while EXP catches up (keeps the p-state
            # clock ramped at 2.4GHz).
            for qh in range(2):
                proj_q(0, qh)
            for sh in range(4):
                proj_k(0, sh)

            # head 0: interleave V projection chunks between score bins
            scores_bin(0, 0)
            scores_bin(0, 1)
            vq = list(range(NKC))
            for g in range(2, NBINS):
                proj_v(vq.pop(0))
                proj_v(vq.pop(0))
                proj_v(vq.pop(0))
                scores_bin(0, g)
            while vq:
                proj_v(vq.pop(0))

            # next-pair projection pieces, split across both heads of the
            # current pair so every stream keeps drain work after its last
            # bin (covers the next head's bin-0 staging-buffer wait)
            for h in range(1, H):
                p = h // 2
                xq = [lambda b=b: av_half(h - 1, b) for b in range(2)]
                pp = p + 1
                if pp < NPAIR:
                    if h == 1:
                        xq += [lambda qh=qh: proj_q(pp, qh) for qh in range(2)]
                        xq += [lambda sh=sh: proj_k(pp, sh) for sh in range(4)]
                    elif h % 2 == 0:
                        xq += [lambda qh=qh: proj_q(pp, qh) for qh in range(2)]
                        xq += [lambda: proj_k(pp, 0)]
                    else:
                        xq += [lambda sh=sh: proj_k(pp, sh) for sh in range(1, 4)]
                scores_bin(h, 0)
                scores_bin(h, 1)
                for g in range(2, NBINS):
                    if xq:
                        xq.pop(0)()
                    scores_bin(h, g)
                while xq:
                    xq.pop(0)()
            av_half(H - 1, 0)
            av_half(H - 1, 1)

            # ---- output projection (bf16) ----
            for i in range(NQT):
                # stg pool: avp's buffers are still pinned by the last
                # head's AV tiles awaiting their DVE normalize chains
                po = stg.tile([P, D], F32, tag="big", padded_shape=[P, BINW])
                for ch in range(4):
                    nc.tensor.matmul(
                        po[:], XT3[:, ch, P * i:P * (i + 1)],
                        wo3[:, ch, :],
                        start=(ch == 0), stop=False)
                nc.tensor.matmul(po[:], on1[0:1, :], bo[0:1, :],
                                 start=False, stop=True)
                ob = ost.tile([P, D], F32, tag="ob")
                nc.vector.tensor_copy(ob[:], po[:])
                nc.sync.dma_start(out_d[P * i:P * (i + 1), :], ob[:])

    nc.compile()
    _cache["nc"] = nc
    return nc


def _host_prep(query, key, value, Wq, bq, Wk, bk, Wv, bv, Wo, bo):
    """Build the 8 per-core input maps (all device-side layouts)."""
    def stack_chmin(W, scale, dt):
        # [H,D,A] -> [128, 16*128], block b=4p+ch: rows 128ch of [W_2p|W_2p+1]
        blocks = []
        for p in range(NPAIR):
            Wp = np.concatenate([W[2 * p], W[2 * p + 1]], axis=1) * scale
            for ch in range(4):
                blocks.append(Wp[P * ch:P * (ch + 1), :])
        return np.stack(blocks, 1).reshape(P, -1).astype(dt)

    def stack_pmin(W, dt):
        # [H,D,A] -> [128, 16*128], block b=4ch+p (moving operand layout)
        blocks = []
        for ch in range(4):
            for p in range(NPAIR):
                Wp = np.concatenate([W[2 * p], W[2 * p + 1]], axis=1)
                blocks.append(Wp[P * ch:P * (ch + 1), :])
        return np.stack(blocks, 1).reshape(P, -1).astype(dt)

    wq_h = stack_chmin(Wq, WSC, E4)
    wk_h = stack_chmin(Wk, WSC, E4)
    wv_h = stack_pmin(Wv, BF)
    wo_h = np.stack([Wo[P * ch:P * (ch + 1), :] for ch in range(4)], 1)
    wo_h = wo_h.reshape(P, -1).astype(BF)

    bq_h = np.stack(
        [np.concatenate([bq[2 * p], bq[2 * p + 1]]) * WSC for p in range(NPAIR)],
        1).astype(np.float32)
    # v-bias adds bv to each head's attention output (softmax rows sum to 1):
    # fold it, with bo, into the single output bias row
    bo_eff = bo + np.concatenate(list(bv)) @ Wo
    bo_h = bo_eff[None, :].astype(BF)
    ones_h = np.ones((1, P), BF)
    kl = np.arange(P)[:, None]
    ql = np.arange(P)[None, :]
    tril_strict = (kl > ql).astype(BF)

    def chunked_T(x, dt):
        # [S', D] -> [128, 4*S'] with col block ch = rows 128ch of x.T
        xT = np.ascontiguousarray(x.T)  # [512, S']
        return xT.reshape(4, P, -1).transpose(1, 0, 2).reshape(P, -1).astype(dt)

    in_maps = []
    for c in range(8):
        b, pair = c // 2, c % 2
        sel = np.concatenate(
            [np.arange(P * (2 * i + pair), P * (2 * i + pair) + P) for i in range(NQT)])
        mE_h = tril_strict if pair == 0 else np.zeros((P, P), BF)
        mO_h = np.ones((P, P), BF) if pair == 0 else tril_strict
        m = {
            "inA": np.concatenate([wq_h, chunked_T(query[b][sel], E4)], 1),
            "inB": np.concatenate([wk_h, chunked_T(key[b], E4)], 1),
            "inC": np.concatenate([wv_h, chunked_T(value[b], BF)], 1),
            "inD": np.concatenate([wo_h, mE_h, mO_h], 1),
            "bq8": bq_h, "bo": bo_h, "ones1": ones_h,
        }
        in_maps.append(m)
    return in_maps


def kernel(query, key, value, Wq, bq, Wk, bk, Wv, bv, Wo, bo):
    from concourse.bass_utils import run_bass_kernel_spmd

    args = [np.asarray(a, dtype=np.float32) for a in
            (query, key, value, Wq, bq, Wk, bk, Wv, bv, Wo, bo)]
    query, key, value, Wq, bq, Wk, bk, Wv, bv, Wo, bo = args

    nc = _build()
    in_maps = _host_prep(*args)
    res = run_bass_kernel_spmd(nc, in_maps, list(range(8)))

    out = np.empty((B, S, D), np.float32)
    for c in range(8):
        b, pair = c // 2, c % 2
        o = res.results[c]["out"]
        for i in range(NQT):
            g = 2 * i + pair
            out[b, P * g:P * (g + 1), :] = o[P * i:P * (i + 1), :]

    # q = S-1 attends to nothing -> reference softmax is uniform over all keys
    for b in range(B):
        vm = value[b].mean(0)
        x = np.concatenate([vm @ Wv[h] + bv[h] for h in range(H)])
        out[b, S - 1, :] = x @ Wo + bo
    return out
